# revision 63
# baseline (speedup 1.0000x reference)
"""Trainium2 Bass kernel for the CudaNorm FastWeight DPFP transformer layer.

Sharding: batch (8) across the 8 cores; each core runs its batch's full layer.

v2 restructure (vs v1): engine-balanced chunk pipeline.
- projection emits q/k feature inputs straight from PSUM into all-head-wide
  strided relu activations (no raw SBUF copy for q/k)
- DPFP feature maps computed for all 16 heads in a handful of wide strided
  DVE/Pool ops (extended xpe layout turns the cyclic rolls into plain shifts)
- Gram matrices (A, S1, S2) packed 4 heads per PSUM bank; masked products and
  kd/dn row-sums are grouped TTs + 3D-AP reduces instead of per-head ops
- delta-rule solve uses the symmetric masked A (U = A . triu) as matmul lhsT
  with per-block cbc broadcasts, eliminating per-head Nt/Bt/B2t tiles and the
  N^2 matmul/transpose
- r carry (cumulative key sum) lives in extra columns of the fast-weight W
  tiles so its kd/dn contributions ride the existing KW/QW matmuls
- output produced time-major, scaled, then pair-packed transposes feed the
  output projection

Self-contained: hardcodes all shapes; host-side prep rearranges weights and
builds masks/identity constants passed as extra DRAM inputs.
"""
import os
import numpy as np
import ml_dtypes

import concourse.bass as bass
import concourse.mybir as mybir
from concourse.bass_utils import run_bass_kernel_spmd
from concourse.tile import TileContext
from concourse.vector_clock import ScopedClock, VectorClock
from contextlib import ExitStack

F32 = mybir.dt.float32
BF16 = mybir.dt.bfloat16
AF = mybir.ActivationFunctionType
OP = mybir.AluOpType
AX = mybir.AxisListType

SLEN, BSZ, DM = 2048, 8, 1024
NH, DH, NROLL = 16, 64, 2
D = 2 * NROLL * DH            # 256 feature dim
C = 128                       # chunk length
NCH = SLEN // C               # 16 chunks
EPS, LN_EPS = 1e-5, 1e-5
SCALE = 1.0 / float(np.sqrt(DH))
OQKV = NH * 192               # 3072
OTOT = OQKV + NH              # 3088 columns of the fused projection
NG = 4                        # head groups
GH = NH // NG                 # heads per group (4)
WBW = 2 * (DH + 1)            # per-head W-tile width: [W_dc0|r_dc0|W_dc1|r_dc1]

# ---------------------------------------------------------------- tile ctx
MAXW = 2


class PatchedTileContext(TileContext):
    """Work around walrus TPB sync-command limits: each instruction carries at
    most 2 sync commands (waits+updates); hoist excess waits onto preceding
    same-engine NoOps (1 wait each), and emit the kernel-tail drain's waits
    one-per-nop on SP."""

    def _lower_ordered_insts(self, ordered):
        for bb_name in list(ordered.keys()):
            new = []
            for inst in ordered[bb_name]:
                si = inst.sync_info
                nupd = len(si.on_update) if si is not None and si.on_update else 0
                maxw = max(0, MAXW - nupd)
                if si is not None and si.on_wait and len(si.on_wait) > maxw:
                    waits = list(si.on_wait)
                    excess = waits if maxw == 0 else waits[:-maxw]
                    keep = [] if maxw == 0 else waits[-maxw:]
                    for w in excess:
                        nop = mybir.InstNoOp(
                            name=self.nc.get_next_instruction_name(),
                            engine=inst.engine, ins=[], outs=[])
                        nop.sync_info = mybir.SyncInfo(on_wait=[w], on_update=[])
                        new.append(nop)
                    inst.sync_info = mybir.SyncInfo(
                        on_wait=keep, on_update=list(si.on_update or []))
                new.append(inst)
            ordered[bb_name] = new
        return super()._lower_ordered_insts(ordered)

    def _drain_and_barrier(self, tick_clock, wait_clock):
        gc = tick_clock.global_clock
        n = len(gc)
        for p in range(n):
            if gc[p] > 0:
                vc = VectorClock([gc[i] if i == p else 0 for i in range(n)])
                nop = self.nc.sync.nop(nofuse=True)
                wait_clock.add_sem_waits(nop.ins, ScopedClock({None: vc}))
        self.nc.sync.drain()
        self.nc.all_engine_barrier()
        assert self.sems is not None
        popped = self.nc._tile_sem_poison_stack.pop()
        assert popped is self._sem_poison
        self.nc.clear_and_free_semaphores(list(self.sems.allocated().values()))
        self.nc.all_engine_barrier()


def _r3(ap, h, w):
    """[128, h*w] -> [128, h, w] view."""
    return ap.rearrange("p (h w) -> p h w", h=h)


def _bc(col_ap, n, w):
    """[128, n] column tile -> [128, n, w] zero-stride broadcast."""
    return col_ap.unsqueeze(2).broadcast_to((128, n, w))


# ---------------------------------------------------------------- program
def build_program(n_chunks=NCH, ln_trivial=False):
    nc = bass.Bass()
    d_hT = nc.declare_dram_parameter("hT", [DM, SLEN], BF16, isOutput=False)
    d_hres = nc.declare_dram_parameter("hres", [SLEN, DM], BF16, isOutput=False)
    d_w = nc.declare_dram_parameter("wqkv", [DM, OTOT], BF16, isOutput=False)
    d_wo = nc.declare_dram_parameter("woT", [DM, DM], BF16, isOutput=False)
    d_lng = nc.declare_dram_parameter("lng", [128, DM], BF16, isOutput=False)
    d_lnb = nc.declare_dram_parameter("lnb", [128, DM], BF16, isOutput=False)
    d_mSU = nc.declare_dram_parameter("maskSU", [128, 128], BF16, isOutput=False)
    d_mUI = nc.declare_dram_parameter("maskUI", [128, 128], BF16, isOutput=False)
    d_id = nc.declare_dram_parameter("identb", [128, 128], BF16, isOutput=False)
    d_out = nc.declare_dram_parameter("out", [SLEN, DM], F32, isOutput=True)

    with PatchedTileContext(nc) as tc, ExitStack() as ctx:
        P = lambda name, bufs, **kw: ctx.enter_context(
            tc.tile_pool(name=name, bufs=bufs, **kw))
        const = P("const", 1)
        state = P("state", 1)
        # streaming SBUF pools
        hts_p = P("hts", 2)
        xpe_p = P("xpe", 2)
        f_p = P("f", 2)           # raw DPFP features, kept until pW
        kqT_p = P("kqT", 2)       # K1T/Q1T (raw, feature-major)
        v_p = P("v", 2)
        col_p = P("col", 2)
        msk_p = P("msk", 2)       # Ug / S1h masked products
        sol_p = P("sol", 2)       # X0/s1/X1/Y + temps (distinct tags)
        outc_p = P("outc", 1)
        outT_p = P("outT", 1)
        xln_p = P("xln", 1)
        # PSUM pools (bank-granular, 8 total): pj(2) gram(2) fs(1) kwq(2) sv(1)
        pj_p = P("pj", 2, space="PSUM")
        gram_p = P("gram", 2, space="PSUM")
        fs_p = P("fs", 1, space="PSUM")
        kwq_p = P("kwq", 2, space="PSUM")
        sv_p = P("sv", 1, space="PSUM")

        # ---- constants
        t_mSU = const.tile([128, 128], BF16, tag="mSU", name="mSU"); nc.sync.dma_start(t_mSU[:], d_mSU[:])
        t_mUI = const.tile([128, 128], BF16, tag="mUI", name="mUI"); nc.sync.dma_start(t_mUI[:], d_mUI[:])
        t_id = const.tile([128, 128], BF16, tag="id", name="id"); nc.sync.dma_start(t_id[:], d_id[:])
        if not ln_trivial:
            t_lng = const.tile([128, DM], BF16, tag="lng", name="lng"); nc.sync.dma_start(t_lng[:], d_lng[:])
            t_lnb = const.tile([128, DM], BF16, tag="lnb", name="lnb"); nc.sync.dma_start(t_lnb[:], d_lnb[:])
        t_ones = const.tile([128, 1], BF16, tag="ones", name="ones")
        nc.vector.memset(t_ones[:], 1.0)
        t_w = []
        for mc in range(8):
            t = const.tile([128, OTOT], BF16, tag=f"w{mc}", name=f"w{mc}")
            nc.sync.dma_start(t[:], d_w[mc * 128:(mc + 1) * 128, :])
            t_w.append(t)
        t_wo = []
        for ic in range(8):
            t = const.tile([128, DM], BF16, tag=f"wo{ic}", name=f"wo{ic}")
            t_wo.append(t)

        # ---- per-group fast-weight state (f32 master + bf16 matmul copy);
        # cols (2h+dc)*65+64 hold the r carry for (head h, dc)
        t_Wm, t_Wb = [], []
        for g in range(NG):
            wm = state.tile([128, GH * WBW], F32, tag=f"wm{g}", name=f"wm{g}")
            nc.vector.memset(wm[:], 0.0)
            wb = state.tile([128, GH * WBW], BF16, tag=f"wb{g}", name=f"wb{g}")
            nc.gpsimd.memset(wb[:], 0.0)
            t_Wm.append(wm); t_Wb.append(wb)

        def head(c):
            cs = slice(c * 128, (c + 1) * 128)
            T = {}
            # ================= projection ==================================
            hts = hts_p.tile([128, 1024], BF16, tag="hts", name="hts")
            for mc in range(8):
                nc.sync.dma_start(hts[:, mc * 128:(mc + 1) * 128],
                                  d_hT[mc * 128:(mc + 1) * 128, cs])
            xpek = xpe_p.tile([128, NH * 130], BF16, tag="xpek", name="xpek")
            xpeq = xpe_p.tile([128, NH * 130], BF16, tag="xpeq", name="xpeq")
            v_all = v_p.tile([128, NH * DH], BF16, tag="v", name="v")
            sig = col_p.tile([128, NH], F32, tag="sig", name="sig")
            for og in range(6):
                o0 = og * 512
                pg = pj_p.tile([128, 512], F32, tag="pj", name="pj")
                for mc in range(8):
                    nc.tensor.matmul(pg[:], hts[:, mc * 128:(mc + 1) * 128],
                                     t_w[mc][:, o0:o0 + 512],
                                     start=(mc == 0), stop=(mc == 7))
                if og < 4:
                    xpe = xpeq if og < 2 else xpek
                    h0 = (og % 2) * 8
                    src = _r3(pg[:], 8, 64)
                    xpe3 = _r3(xpe[:], NH, 130)
                    nc.scalar.activation(xpe3[:, h0:h0 + 8, 2:66], src, AF.Relu)
                    nc.scalar.activation(xpe3[:, h0:h0 + 8, 66:130], src,
                                         AF.Relu, scale=-1.0)
                else:
                    nc.scalar.mul(v_all[:, (og - 4) * 512:(og - 3) * 512], pg[:], 1.0)
            pgb = pj_p.tile([128, NH], F32, tag="pj", name="pj")
            for mc in range(8):
                nc.tensor.matmul(pgb[:], hts[:, mc * 128:(mc + 1) * 128],
                                 t_w[mc][:, OQKV:OTOT],
                                 start=(mc == 0), stop=(mc == 7))
            nc.scalar.activation(sig[:], pgb[:], AF.Sigmoid)

            # ================= raw DPFP features (all heads) ================
            fk = f_p.tile([128, NH * D], BF16, tag="fk", name="fk")
            fq = f_p.tile([128, NH * D], BF16, tag="fq", name="fq")
            for xpe, f, eng in ((xpek, fk, nc.vector), (xpeq, fq, nc.gpsimd)):
                xpe3 = _r3(xpe[:], NH, 130)
                f3 = _r3(f[:], NH, D)
                nc.gpsimd.tensor_copy(xpe3[:, :, 0:2], xpe3[:, :, 128:130])
                eng.tensor_tensor(f3[:, :, 0:128], xpe3[:, :, 2:130],
                                  xpe3[:, :, 1:129], OP.mult)
                eng.tensor_tensor(f3[:, :, 128:256], xpe3[:, :, 2:130],
                                  xpe3[:, :, 0:128], OP.mult)

            # ================= feature transposes (raw) =====================
            K1T = kqT_p.tile([128, 32 * 128], BF16, tag="K1T", name="K1T")
            Q1T = kqT_p.tile([128, 32 * 128], BF16, tag="Q1T", name="Q1T")
            pFS = fs_p.tile([128, 32], F32, tag="fs", name="pFS")
            for side, (src, dst) in enumerate(((fk, K1T), (fq, Q1T))):
                for t4 in range(8):
                    pt = gram_p.tile([128, 512], BF16, tag="gram", name="tp")
                    for j in range(4):
                        b = t4 * 4 + j
                        nc.tensor.transpose(pt[:, j * 128:(j + 1) * 128],
                                            src[:, b * 128:(b + 1) * 128], t_id[:])
                    nc.scalar.mul(dst[:, t4 * 512:(t4 + 1) * 512], pt[:], 1.0)
                # fs matmuls for this side fill the other side's evac waits
                for h in range(NH):
                    for dc in range(2):
                        nc.tensor.matmul(pFS[:, side * 16 + h:side * 16 + h + 1],
                                         dst[:, (2 * h + dc) * 128:(2 * h + dc + 1) * 128],
                                         t_ones[:], start=(dc == 0), stop=(dc == 1))
            frec = col_p.tile([128, 32], F32, tag="frec", name="frec")
            nc.vector.reciprocal(frec[:], pFS[:])
            dkb = col_p.tile([128, 32], BF16, tag="dkb", name="dkb")
            nc.vector.tensor_copy(dkb[:], frec[:])
            outT_all = outT_p.tile([128, 8 * 128], BF16, tag="oT", name="oT")
            T.update(fk=fk, fq=fq, K1T=K1T, Q1T=Q1T, v_all=v_all, sig=sig,
                     frec=frec, dkb=dkb, outT=outT_all)
            return T

        def tail(c, T):
            cs = slice(c * 128, (c + 1) * 128)
            outT_all = T["outT"]
            hr = xln_p.tile([128, DM], BF16, tag="hr", name="hr")
            nc.sync.dma_start(hr[:], d_hres[cs, :])
            x = xln_p.tile([128, DM], BF16, tag="x", name="x")
            xs2 = col_p.tile([128, 2], F32, tag="xs2", name="xs2")
            for og in range(2):
                pAT = gram_p.tile([128, 512], F32, tag="gram", name="pAT")
                for ic in range(8):
                    nc.tensor.matmul(pAT[:], outT_all[:, ic * 128:(ic + 1) * 128],
                                     t_wo[ic][:, og * 512:(og + 1) * 512],
                                     start=(ic == 0), stop=False)
                # residual add rides the accumulation via identity matmul
                nc.tensor.matmul(pAT[:], t_id[:],
                                 hr[:, og * 512:(og + 1) * 512],
                                 start=False, stop=True)
                nc.scalar.activation(x[:, og * 512:(og + 1) * 512], pAT[:],
                                     AF.Identity, accum_out=xs2[:, og:og + 1])
            xsum = col_p.tile([128, 1], F32, tag="xsum", name="xsum")
            nc.vector.tensor_tensor(xsum[:], xs2[:, 0:1], xs2[:, 1:2], OP.add)
            nmu = col_p.tile([128, 1], F32, tag="nmu", name="nmu")
            nc.vector.tensor_scalar_mul(nmu[:], xsum[:], -1.0 / DM)
            # var = E[x^2] - mu^2 (no centering pass needed)
            vscr = xln_p.tile([128, DM], BF16, tag="vscr", name="vscr")
            var = col_p.tile([128, 1], F32, tag="var", name="var")
            nc.scalar.activation(vscr[:], x[:], AF.Square, accum_out=var[:])
            mu2 = col_p.tile([128, 1], F32, tag="mu2", name="mu2")
            nc.vector.tensor_tensor(mu2[:], nmu[:], nmu[:], OP.mult)
            vare = col_p.tile([128, 1], F32, tag="vare", name="vare")
            nc.vector.tensor_scalar(vare[:], var[:], 1.0 / DM, float(LN_EPS),
                                    OP.mult, OP.add)
            vare2 = col_p.tile([128, 1], F32, tag="vare2", name="vare2")
            nc.vector.tensor_tensor(vare2[:], vare[:], mu2[:], OP.subtract)
            sd = col_p.tile([128, 1], F32, tag="sd", name="sd")
            nc.scalar.sqrt(sd[:], vare2[:])
            rstd = col_p.tile([128, 1], F32, tag="rstd", name="rstd")
            nc.vector.reciprocal(rstd[:], sd[:])
            nmurs = col_p.tile([128, 1], F32, tag="nmurs", name="nmurs")
            nc.vector.tensor_tensor(nmurs[:], nmu[:], rstd[:], OP.mult)
            xout = xln_p.tile([128, DM], F32, tag="xout", name="xout")
            # (x - mu) * rstd in one Act op: Identity(x*rstd + (-mu*rstd))
            nc.scalar.activation(xout[:], x[:], AF.Identity,
                                 bias=nmurs[:], scale=rstd[:])
            if not ln_trivial:
                nc.vector.scalar_tensor_tensor(xout[:], xout[:], 1.0, t_lng[:],
                                               OP.mult, OP.mult)
                nc.vector.tensor_tensor(xout[:], xout[:], t_lnb[:], OP.add)
            nc.sync.dma_start(d_out[cs, :], xout[:])

        cur = head(0)
        # W_o loads deferred behind chunk-0 inputs (needed only at tail(0))
        for ic in range(8):
            nc.sync.dma_start(t_wo[ic][:], d_wo[ic * 128:(ic + 1) * 128, :])
        for c in range(n_chunks):
            T = cur
            K1T = T["K1T"]; Q1T = T["Q1T"]; fk = T["fk"]; fq = T["fq"]
            v_all = T["v_all"]; sig = T["sig"]; frec = T["frec"]; dkb = T["dkb"]
            outT_all = T["outT"]
            stash = {}
            def stage1(g):
                hg = [g * GH + i for i in range(GH)]
                Wb = t_Wb[g]
                # Grams: A (k.k), S1 (k_s.q_t) in raw space
                pA = gram_p.tile([128, 512], F32, tag="gram", name="pA")
                pS1 = gram_p.tile([128, 512], F32, tag="gram", name="pS1")
                for i, h in enumerate(hg):
                    for dc in range(2):
                        kb = K1T[:, (2 * h + dc) * 128:(2 * h + dc + 1) * 128]
                        qb = Q1T[:, (2 * h + dc) * 128:(2 * h + dc + 1) * 128]
                        st = (dc == 0); sp = (dc == 1)
                        sl = slice(i * 128, (i + 1) * 128)
                        nc.tensor.matmul(pA[:, sl], kb, kb, start=st, stop=sp)
                        nc.tensor.matmul(pS1[:, sl], kb, qb, start=st, stop=sp)
                # KW / QW (64-wide blocks share one bank-tile)
                pKWO = kwq_p.tile([128, 512], F32, tag="kwq", name="pKWO")
                for i, h in enumerate(hg):
                    for dc in range(2):
                        kb = K1T[:, (2 * h + dc) * 128:(2 * h + dc + 1) * 128]
                        qb = Q1T[:, (2 * h + dc) * 128:(2 * h + dc + 1) * 128]
                        wsl = Wb[:, (2 * i + dc) * 65:(2 * i + dc) * 65 + 64]
                        nc.tensor.matmul(pKWO[:, i * 64:(i + 1) * 64], kb, wsl,
                                         start=(dc == 0), stop=(dc == 1))
                        nc.tensor.matmul(pKWO[:, 256 + i * 64:256 + (i + 1) * 64],
                                         qb, wsl, start=(dc == 0), stop=False)
                # masked products (bf16, SBUF)
                Ug = msk_p.tile([128, 512], BF16, tag="U", name="Ug")
                S1h = msk_p.tile([128, 512], BF16, tag="S1h", name="S1h")
                mSU_b = t_mSU[:].unsqueeze(1).broadcast_to((128, GH, 128))
                mUI_b = t_mUI[:].unsqueeze(1).broadcast_to((128, GH, 128))
                nc.vector.tensor_tensor(_r3(Ug[:], GH, 128), _r3(pA[:], GH, 128),
                                        mSU_b, OP.mult)
                nc.vector.tensor_tensor(_r3(S1h[:], GH, 128), _r3(pS1[:], GH, 128),
                                        mUI_b, OP.mult)
                # kd/dn: dk-weighted column sums + r terms, all on PE
                pKD = sv_p.tile([128, 2 * GH], F32, tag="sv", name="pKD")
                for i, h in enumerate(hg):
                    dkc = dkb[:, g * GH + i:g * GH + i + 1]
                    nc.tensor.matmul(pKD[:, i:i + 1],
                                     Ug[:, i * 128:(i + 1) * 128], dkc,
                                     start=True, stop=False)
                    nc.tensor.matmul(pKD[:, GH + i:GH + i + 1],
                                     S1h[:, i * 128:(i + 1) * 128], dkc,
                                     start=True, stop=False)
                    for dc in range(2):
                        kb = K1T[:, (2 * h + dc) * 128:(2 * h + dc + 1) * 128]
                        qb = Q1T[:, (2 * h + dc) * 128:(2 * h + dc + 1) * 128]
                        rsl = Wb[:, (2 * i + dc) * 65 + 64:(2 * i + dc) * 65 + 65]
                        nc.tensor.matmul(pKD[:, i:i + 1], kb, rsl,
                                         start=False, stop=(dc == 1))
                        nc.tensor.matmul(pKD[:, GH + i:GH + i + 1], qb, rsl,
                                         start=False, stop=(dc == 1))
                stash[g] = (pKWO, Ug, S1h, pKD)

            def stage2(sg):
                # supergroup sg covers groups g0=2*sg, g1=2*sg+1 (8 heads)
                g0, g1 = 2 * sg, 2 * sg + 1
                SH = 2 * GH                      # 8 heads
                h0 = g0 * GH                     # first head index
                dk8 = frec[:, h0:h0 + SH]
                dq8 = frec[:, 16 + h0:16 + h0 + SH]
                pKWO0, Ug0, S1h0, pKD0 = stash.pop(g0)
                pKWO1, Ug1, S1h1, pKD1 = stash.pop(g1)
                pKWO_ = (pKWO0, pKWO1); Ug_ = (Ug0, Ug1); S1h_ = (S1h0, S1h1)
                pKD_ = (pKD0, pKD1)
                # columns (all [128, SH] f32, halves filled per source group)
                CT = lambda nm: col_p.tile([128, SH], F32, tag=nm, name=nm)
                kdf = CT("kdf")
                for j in range(2):
                    nc.vector.tensor_tensor(kdf[:, j * GH:(j + 1) * GH],
                                            pKD_[j][:, 0:GH],
                                            dk8[:, j * GH:(j + 1) * GH], OP.mult)
                if c == 0:
                    nc.vector.memset(kdf[0:1, :], 1.0)
                dnf = CT("dnf")
                for j in range(2):
                    nc.vector.tensor_tensor(dnf[:, j * GH:(j + 1) * GH],
                                            pKD_[j][:, GH:2 * GH],
                                            dq8[:, j * GH:(j + 1) * GH], OP.mult)
                dne = CT("dne"); nc.vector.tensor_scalar_add(dne[:], dnf[:], EPS)
                dnr = CT("dnr"); nc.vector.reciprocal(dnr[:], dne[:])
                dnrS = CT("dnrS"); nc.vector.tensor_scalar_mul(dnrS[:], dnr[:], SCALE)
                dnrQ = CT("dnrQ"); nc.vector.tensor_tensor(dnrQ[:], dnrS[:], dq8, OP.mult)
                ceps = CT("ceps"); nc.vector.tensor_scalar_add(ceps[:], kdf[:], EPS)
                c4 = CT("c4"); nc.vector.reciprocal(c4[:], ceps[:])
                t0 = CT("t0"); nc.vector.tensor_tensor(t0[:], kdf[:], c4[:], OP.mult)
                cb4 = CT("cb4")
                nc.vector.tensor_tensor(cb4[:], t0[:], sig[:, h0:h0 + SH], OP.mult)
                cbc4 = CT("cbc4"); nc.vector.tensor_tensor(cbc4[:], cb4[:], c4[:], OP.mult)
                dk2 = CT("dk2"); nc.vector.tensor_tensor(dk2[:], dk8, dk8, OP.mult)
                cbV = CT("cbV"); nc.vector.tensor_tensor(cbV[:], cb4[:], dk8, OP.mult)
                e4 = CT("e4"); nc.vector.tensor_tensor(e4[:], cbc4[:], dk2[:], OP.mult)
                # X0' = cbV*V - e*KWr  (8 heads wide)
                vcb = sol_p.tile([128, SH * DH], BF16, tag="vcb", name="vcb")
                nc.vector.tensor_tensor(
                    _r3(vcb[:], SH, DH),
                    _r3(v_all[:, h0 * DH:(h0 + SH) * DH], SH, DH),
                    _bc(cbV[:], SH, DH), OP.mult)
                tkw = sol_p.tile([128, SH * DH], BF16, tag="tkw", name="tkw")
                for j in range(2):
                    nc.vector.tensor_tensor(
                        _r3(tkw[:, j * 256:(j + 1) * 256], GH, DH),
                        _r3(pKWO_[j][:, 0:256], GH, DH),
                        _bc(e4[:, j * GH:(j + 1) * GH], GH, DH), OP.mult)
                X0 = sol_p.tile([128, SH * DH], BF16, tag="X0", name="X0")
                nc.vector.tensor_tensor(X0[:], vcb[:], tkw[:], OP.subtract)
                # solve Y' = (I-N)(I+N^2)X0', scalar e = cbc*dk^2
                def usolve(dst, src):
                    for j in range(2):
                        for i in range(GH):
                            nc.tensor.matmul(
                                dst[:, (j * GH + i) * DH:(j * GH + i + 1) * DH],
                                Ug_[j][:, i * 128:(i + 1) * 128],
                                src[:, (j * GH + i) * DH:(j * GH + i + 1) * DH],
                                start=True, stop=True)
                # 3-term Neumann: Y = X0 - N X0 + N^2 X0  (N = diag(e) Mr)
                p1 = sv_p.tile([128, SH * DH], F32, tag="sv", name="p1")
                usolve(p1, X0)
                s1 = sol_p.tile([128, SH * DH], BF16, tag="s1", name="s1")
                nc.vector.tensor_tensor(_r3(s1[:], SH, DH), _r3(p1[:], SH, DH),
                                        _bc(e4[:], SH, DH), OP.mult)
                xm = sol_p.tile([128, SH * DH], BF16, tag="X1", name="xm")
                nc.vector.tensor_tensor(xm[:], X0[:], s1[:], OP.subtract)
                p2 = sv_p.tile([128, SH * DH], F32, tag="sv", name="p2")
                usolve(p2, s1)
                t2 = sol_p.tile([128, SH * DH], BF16, tag="tx", name="t2")
                nc.vector.tensor_tensor(_r3(t2[:], SH, DH), _r3(p2[:], SH, DH),
                                        _bc(e4[:], SH, DH), OP.mult)
                # Yt extended to 65-wide blocks; col 64 = dk (rides pW matmuls
                # to produce this chunk's r delta)
                Yt = sol_p.tile([128, SH * 65], BF16, tag="Yt", name="Yt")
                y3o = _r3(Yt[:], SH, 65)[:, :, 0:64]
                nc.vector.tensor_tensor(y3o, _r3(xm[:], SH, DH), _r3(t2[:], SH, DH),
                                        OP.add)
                nc.gpsimd.tensor_copy(
                    _r3(Yt[:], SH, 65)[:, :, 64:65].squeeze(2), dkb[:, h0:h0 + SH])
                # fast weight update: W += Fk^T Y' ; r += Fk^T dk
                for half in range(4):
                    g = g0 + half // 2
                    Wm = t_Wm[g]
                    pW = kwq_p.tile([128, 2 * 130], F32, tag="kwq", name="pW")
                    for ii in range(2):
                        i = half * 2 + ii          # head within supergroup
                        h = h0 + i
                        for dc in range(2):
                            nc.tensor.matmul(
                                pW[:, (2 * ii + dc) * 65:(2 * ii + dc + 1) * 65],
                                fk[:, (2 * h + dc) * 128:(2 * h + dc + 1) * 128],
                                Yt[:, i * 65:(i + 1) * 65],
                                start=True, stop=True)
                    wsl = Wm[:, (half % 2) * 260:(half % 2 + 1) * 260]
                    nc.vector.tensor_tensor(wsl, pW[:], wsl, OP.add)
                nc.gpsimd.tensor_copy(t_Wb[g0][:], t_Wm[g0][:])
                nc.gpsimd.tensor_copy(t_Wb[g1][:], t_Wm[g1][:])
                # attention part into pKWO QW regions (closes those groups)
                for j in range(2):
                    for i in range(GH):
                        nc.tensor.matmul(
                            pKWO_[j][:, 256 + i * 64:256 + (i + 1) * 64],
                            S1h_[j][:, i * 128:(i + 1) * 128],
                            Yt[:, (j * GH + i) * 65:(j * GH + i) * 65 + 64],
                            start=False, stop=True)
                # scaled output
                outc = outc_p.tile([128, SH * DH], BF16, tag="outc", name="outc")
                for j in range(2):
                    nc.vector.tensor_tensor(
                        _r3(outc[:, j * 256:(j + 1) * 256], GH, DH),
                        _r3(pKWO_[j][:, 256:512], GH, DH),
                        _bc(dnrQ[:, j * GH:(j + 1) * GH], GH, DH), OP.mult)
                # transpose output pairs for the output projection
                for p in range(4):
                    pT = sv_p.tile([128, 128], BF16, tag="sv", name="pT")
                    for q in range(2):
                        base = q * 64
                        nc.tensor.transpose(pT[base:base + 64, :],
                                            outc[:, (2 * p + q) * 64:(2 * p + q + 1) * 64],
                                            t_id[:], tile_position=(0, base))
                    nc.scalar.mul(
                        outT_all[:, (sg * 4 + p) * 128:(sg * 4 + p + 1) * 128],
                        pT[:], 1.0)


            stage1(0); stage1(1)
            stage2(0)
            stage1(2); stage1(3)
            if c + 1 < n_chunks:
                cur = head(c + 1)
            stage2(1)
            tail(c, T)

    return nc


# ---------------------------------------------------------------- host side
def _prep_core_inputs(h_b, wq, woT, lng, lnb, masks):
    bf16 = ml_dtypes.bfloat16
    hT = np.ascontiguousarray(h_b.T).astype(bf16)                  # [1024, 2048]
    hres = h_b.astype(bf16)
    out = {"hT": hT, "hres": hres, "wqkv": wq, "woT": woT,
           "lng": lng, "lnb": lnb}
    out.update(masks)
    return out


_cached = {}


def kernel(h, W_qkvb, W_o, ln_g, ln_b):
    bf16 = ml_dtypes.bfloat16
    h = np.asarray(h, np.float32)
    W_qkvb = np.asarray(W_qkvb, np.float32)
    W_o = np.asarray(W_o, np.float32)
    ln_g = np.asarray(ln_g, np.float32)
    ln_b = np.asarray(ln_b, np.float32)
    ln_trivial = bool(np.all(ln_g == 1.0) and np.all(ln_b == 0.0))
    key = ("nc", ln_trivial)
    if key not in _cached:
        _cached[key] = build_program(ln_trivial=ln_trivial)
        _cached["nc"] = _cached[key]
    nc = _cached[key]

    Wr = W_qkvb.reshape(NH, 193, DM)
    wq = np.empty((DM, OTOT), dtype=bf16)
    wq[:, 0:1024] = Wr[:, 0:64, :].reshape(NH * 64, DM).T
    wq[:, 1024:2048] = Wr[:, 64:128, :].reshape(NH * 64, DM).T
    wq[:, 2048:3072] = Wr[:, 128:192, :].reshape(NH * 64, DM).T
    wq[:, 3072:3088] = Wr[:, 192, :].T
    woT = np.ascontiguousarray(W_o.T).astype(bf16)
    lng = np.broadcast_to(ln_g[None, :], (128, DM)).astype(bf16).copy()
    lnb = np.broadcast_to(ln_b[None, :], (128, DM)).astype(bf16).copy()
    ii, jj = np.indices((128, 128))
    masks = {
        "maskSU": (jj > ii).astype(bf16),
        "maskUI": (jj >= ii).astype(bf16),
        "identb": np.eye(128, dtype=bf16),
    }
    in_maps = [_prep_core_inputs(h[:, b, :], wq, woT, lng, lnb, masks)
               for b in range(BSZ)]
    res = run_bass_kernel_spmd(nc, in_maps, list(range(BSZ)),
                               trace=os.environ.get("BASS_TRACE", "") == "1")
    out = np.stack([res.results[b]["out"] for b in range(BSZ)], axis=1)
    kernel.last_exec_time_ns = res.exec_time_ns
    return out.astype(np.float32)
